# revision 14
# baseline (speedup 1.0000x reference)
"""GRU decoder (nn_Decoder) Trainium2 Bass kernel — fp8 DoubleRow edition.

Strategy: pure data parallelism — batch B=8192 over 8 cores (1024 rows each),
weights replicated. Features on partitions, batch on the free axis.

Per core, per GRU step (batch chunks of N=256):
  - All gate matmuls are fp8e4 DoubleRow (0.5 cy/row): recurrent W_hh.T
    (scaled by S, quantized to fp8) against h8 (fp8 copy of h), plus one-hot
    token matmuls for the r/z input gates (table pre-scaled by S, biases
    folded into a const-1 row). The n-gate input row is gathered on GPSIMD
    from a [128,2,A] table (pre-scaled by S, b_ih_n folded); b_hh_n enters
    the phn PSUM via a const-row matmul so npre is a plain tensor_tensor.
  - Act: sigmoid over the merged [128,4,256] r/z PSUM (scale=1/S), tanh over
    the SBUF t2 tensor (scale=1/S), one table, no reloads.
  - GPSIMD: npre = phn*r (stt, PSUM in), token gathers.
  - DVE: t2 = npre + gin, combine h' = n + z*(h-n), h8 convert, p1 relu.
  - Projections p1/p2 fp8/bf16 with batch-packed PSUM [128, 512]; logits
    (+bp2 via a const-row matmul) DMA'd straight from PSUM to DRAM.
"""

import numpy as np
import ml_dtypes

B, L, H, A, T, E = 8192, 128, 256, 32, 65, 8
NCORES = 8
BC = B // NCORES          # 1024 batch rows per core
NCH = 4                   # batch chunks per step
CH = BC // NCH            # 256
G3 = 3 * H                # 768
S = 16.0                  # fp8 table scale
KT = 17                   # one-hot k-tile height (2*17 = 34 = A + const + pad)

BF16 = ml_dtypes.bfloat16
FP8 = ml_dtypes.float8_e4m3fn

_CACHE = {}


def _build(trace=False, debug=False):
    import concourse.bass as bass
    import concourse.bacc as bacc
    import concourse.tile as tile
    from concourse import mybir
    from contextlib import ExitStack

    f32 = mybir.dt.float32
    bf16 = mybir.dt.bfloat16
    fp8 = mybir.dt.float8e4
    u16 = mybir.dt.uint16
    Alu = mybir.AluOpType
    Act = mybir.ActivationFunctionType
    DR = mybir.MatmulPerfMode.DoubleRow

    nc = bacc.Bacc("TRN2", target_bir_lowering=False, debug=False)

    lat = nc.dram_tensor("lat", [128, BC], bf16, kind="ExternalInput")
    oh = nc.dram_tensor("oh", [T, KT, 2, BC], fp8, kind="ExternalInput")
    whh = nc.dram_tensor("whh", [128, 2, 6, 128], fp8, kind="ExternalInput")
    giv = nc.dram_tensor("giv", [KT, 2, 4, 128], fp8, kind="ExternalInput")
    cb8 = nc.dram_tensor("cb8", [KT, 2, 2, 128], fp8, kind="ExternalInput")
    wd0 = nc.dram_tensor("wd0", [128, 2, 128], bf16, kind="ExternalInput")
    wd1 = nc.dram_tensor("wd1", [128, 2, 2, 128], bf16, kind="ExternalInput")
    wd2 = nc.dram_tensor("wd2", [128, 2, 2, 128], bf16, kind="ExternalInput")
    wp18 = nc.dram_tensor("wp18", [128, 2, A], fp8, kind="ExternalInput")
    wp2 = nc.dram_tensor("wp2", [128, A], bf16, kind="ExternalInput")
    bp2r = nc.dram_tensor("bp2r", [1, A], bf16, kind="ExternalInput")
    bias = nc.dram_tensor("bias", [128, 8], f32, kind="ExternalInput")
    tokw = nc.dram_tensor("tokw", [128, T, 64], u16, kind="ExternalInput")
    gtab = nc.dram_tensor("gtab", [128, 2, A], bf16, kind="ExternalInput")
    out = nc.dram_tensor("out", [BC, T, A], f32, kind="ExternalOutput")
    if debug:
        dbg_rz = nc.dram_tensor("dbg_rz", [128, 4, BC], f32, kind="ExternalOutput")
        dbg_npre = nc.dram_tensor("dbg_npre", [128, 2, BC], f32, kind="ExternalOutput")
        dbg_t2 = nc.dram_tensor("dbg_t2", [128, 2, BC], f32, kind="ExternalOutput")
        dbg_h1 = nc.dram_tensor("dbg_h1", [128, 2, BC], f32, kind="ExternalOutput")
        dbg_h0 = nc.dram_tensor("dbg_h0", [128, 2, BC], f32, kind="ExternalOutput")
        dbg_gin = nc.dram_tensor("dbg_gin", [128, 2, BC], f32, kind="ExternalOutput")
        dbg_p1 = nc.dram_tensor("dbg_p1", [A, BC], f32, kind="ExternalOutput")
        dbg_pr = nc.dram_tensor("dbg_pr", [128, 4, BC], f32, kind="ExternalOutput")
        dbg_ph = nc.dram_tensor("dbg_ph", [128, 2, BC], f32, kind="ExternalOutput")

    outv = out.rearrange("(j p) t a -> p j t a", j=8, p=128)

    with ExitStack() as ctx:
        tc = ctx.enter_context(tile.TileContext(nc))
        const = ctx.enter_context(tc.tile_pool(name="const", bufs=1))
        hp = ctx.enter_context(tc.tile_pool(name="hp", bufs=2))
        work = ctx.enter_context(tc.tile_pool(name="work", bufs=2))
        ohp = ctx.enter_context(tc.tile_pool(name="ohp", bufs=3))
        psum = ctx.enter_context(tc.tile_pool(name="psum", bufs=1, space="PSUM"))

        # ---- constants ----
        lat_sb = const.tile([128, BC], bf16, tag="lat")
        nc.sync.dma_start(out=lat_sb[:], in_=lat[:])
        whh_sb = const.tile([128, 2, 6, 128], fp8, tag="whh")
        nc.sync.dma_start(out=whh_sb[:], in_=whh[:])
        giv_sb = const.tile([KT, 2, 4, 128], fp8, tag="giv")
        nc.sync.dma_start(out=giv_sb[:], in_=giv[:])
        cb8_sb = const.tile([KT, 2, 2, 128], fp8, tag="cb8")
        nc.sync.dma_start(out=cb8_sb[:], in_=cb8[:])
        wd0_sb = const.tile([128, 2, 128], bf16, tag="wd0")
        nc.sync.dma_start(out=wd0_sb[:], in_=wd0[:])
        wd1_sb = const.tile([128, 2, 2, 128], bf16, tag="wd1")
        nc.sync.dma_start(out=wd1_sb[:], in_=wd1[:])
        wd2_sb = const.tile([128, 2, 2, 128], bf16, tag="wd2")
        nc.sync.dma_start(out=wd2_sb[:], in_=wd2[:])
        wp1_sb = const.tile([128, 2, A], fp8, tag="wp18")
        nc.sync.dma_start(out=wp1_sb[:], in_=wp18[:])
        wp2_sb = const.tile([128, A], bf16, tag="wp2")
        nc.sync.dma_start(out=wp2_sb[:], in_=wp2[:])
        bp2_sb = const.tile([1, A], bf16, tag="bp2r")
        nc.sync.dma_start(out=bp2_sb[:], in_=bp2r[:])
        one_sb = const.tile([1, 128], bf16, tag="one")
        nc.vector.memset(one_sb[:], 1.0)
        bias_sb = const.tile([128, 8], f32, tag="bias")
        nc.sync.dma_start(out=bias_sb[:], in_=bias[:])
        tokw_sb = const.tile([128, T, 64], u16, tag="tokw")
        nc.sync.dma_start(out=tokw_sb[:], in_=tokw[:])
        gtab_sb = const.tile([128, 2, A], bf16, tag="gtab")
        nc.sync.dma_start(out=gtab_sb[:], in_=gtab[:])

        # ---- MLP prologue: h0 = relu(relu(lat@Wd0+b)@Wd1+b)@Wd2+b ----
        # bf16 matmuls, feature-on-partition layout, chunked by CH columns.
        h1 = work.tile([128, 2, BC], bf16, tag="mlp1", name="mlp1")
        for c in range(NCH):
            cs = slice(c * CH, (c + 1) * CH)
            ps = psum.tile([128, 2, CH], f32, tag="ph", bufs=1, name=f"mlp1p_{c}")
            for m in range(2):
                nc.tensor.matmul(ps[:, m, :], wd0_sb[:, m, :], lat_sb[:, cs],
                                 start=True, stop=True)
            for m in range(2):
                nc.scalar.activation(out=h1[:, m, cs], in_=ps[:, m, :],
                                     func=Act.Relu, bias=bias_sb[:, m:m + 1])
        h2 = work.tile([128, 2, BC], bf16, tag="mlp2", name="mlp2")
        for c in range(NCH):
            cs = slice(c * CH, (c + 1) * CH)
            ps = psum.tile([128, 2, CH], f32, tag="ph", bufs=1, name=f"mlp2p_{c}")
            for m in range(2):
                for kc in range(2):
                    nc.tensor.matmul(ps[:, m, :], wd1_sb[:, kc, m, :],
                                     h1[:, kc, cs], start=(kc == 0), stop=(kc == 1))
            for m in range(2):
                nc.scalar.activation(out=h2[:, m, cs], in_=ps[:, m, :],
                                     func=Act.Relu, bias=bias_sb[:, 2 + m:3 + m])
        hbf = hp.tile([128, 2, BC], bf16, tag="hbf", name="h0bf")
        h8 = hp.tile([128, 2, BC], fp8, tag="h8", name="h0f8")
        for c in range(NCH):
            cs = slice(c * CH, (c + 1) * CH)
            ps = psum.tile([128, 2, CH], f32, tag="ph", bufs=1, name=f"mlp3p_{c}")
            for m in range(2):
                for kc in range(2):
                    nc.tensor.matmul(ps[:, m, :], wd2_sb[:, kc, m, :],
                                     h2[:, kc, cs], start=(kc == 0), stop=(kc == 1))
            for m in range(2):
                nc.scalar.activation(out=hbf[:, m, cs], in_=ps[:, m, :],
                                     func=Act.Identity, bias=bias_sb[:, 4 + m:5 + m])
        nc.vector.tensor_scalar(out=h8[:], in0=hbf[:], scalar1=1.0,
                                scalar2=None, op0=Alu.mult)

        # ---- GRU steps ----
        def emit_proj(h8_t, tp):
            """Output projections for step tp from the fp8 hidden state."""
            p1ps = psum.tile([A, BC], f32, tag="p1", bufs=1, name=f"p1ps_{tp}")
            for c in range(NCH):
                nc.tensor.matmul(
                    p1ps[:, c * CH:(c + 1) * CH], wp1_sb[:],
                    h8_t[:, :, c * CH:(c + 1) * CH],
                    start=True, stop=True, perf_mode=DR,
                )
            p1bf = work.tile([A, BC], bf16, tag="p1bf", name=f"p1bf_{tp}")
            nc.scalar.activation(out=p1bf[:], in_=p1ps[:], func=Act.Relu,
                                 bias=bias_sb[0:A, 6:7])
            if debug and tp == 0:
                p1st = const.tile([A, BC], f32, tag="tapp1", name="tapp1")
                nc.vector.tensor_scalar(out=p1st[:], in0=p1bf[:], scalar1=1.0,
                                        scalar2=None, op0=Alu.mult)
                nc.sync.dma_start(out=dbg_p1[:], in_=p1st[:])
            p2v = psum.tile([128, 8, A], f32, tag="p2", bufs=1, name=f"p2ps_{tp}")
            for j in range(8):
                nc.tensor.matmul(
                    p2v[:, j, :], one_sb[:], bp2_sb[:],
                    start=True, stop=False, tile_position=(0, 0),
                )
                nc.tensor.matmul(
                    p2v[:, j, :], p1bf[:, 128 * j:128 * (j + 1)], wp2_sb[0:A, :],
                    start=False, stop=True,
                )
            outsb = work.tile([128, 8, A], f32, tag="outsb", name=f"outsb_{tp}")
            nc.scalar.activation(out=outsb[:], in_=p2v[:], func=Act.Copy)
            nc.sync.dma_start(out=outv[:, :, tp, :], in_=outsb[:])

        for t in range(T):
            oh_t = ohp.tile([KT, 2, BC], fp8, tag="oh", name=f"oh_{t}")
            nc.sync.dma_start(out=oh_t[:], in_=oh[t])
            # n-gate input rows gathered by token on GPSIMD (S-scaled table)
            gin = work.tile([128, 2, BC], bf16, tag="gin", name=f"gin_{t}")
            for m in range(2):
                nc.gpsimd.indirect_copy(
                    out=gin[:, m, :], data=gtab_sb[:, m, :],
                    idxs=tokw_sb[:, t, :],
                    i_know_ap_gather_is_preferred=True,
                )

            rzbf = work.tile([128, 4, BC], bf16, tag="rzbf", name=f"rzbf_{t}")
            npre = work.tile([128, 2, BC], bf16, tag="npre", name=f"npre_{t}")
            t2 = work.tile([128, 2, BC], bf16, tag="t2", name=f"t2_{t}")
            nbf = work.tile([128, 2, BC], bf16, tag="nbf", name=f"nbf_{t}")
            dbf = work.tile([128, 2, BC], bf16, tag="dbf", name=f"dbf_{t}")
            ebf = work.tile([128, 2, BC], bf16, tag="ebf", name=f"ebf_{t}")
            hbf_new = hp.tile([128, 2, BC], bf16, tag="hbf", name=f"hbf_{t}")
            h8_new = hp.tile([128, 2, BC], fp8, tag="h8", name=f"h8_{t}")

            for c in range(NCH):
                cs = slice(c * CH, (c + 1) * CH)
                rzp = psum.tile([128, 4, CH], f32, tag="rz", bufs=2,
                                name=f"rzp_{t}_{c}")
                php = psum.tile([128, 2, CH], f32, tag="ph", bufs=1,
                                name=f"php_{t}_{c}")
                # NB: start=True arms a zero-on-next-write over the whole 2KB
                # psum bank, so each region's start..stop group must complete
                # before the next start in the same bank.
                for j in range(4):
                    nc.tensor.matmul(rzp[:, j, :], giv_sb[:, :, j, :],
                                     oh_t[:, :, cs], start=True, stop=False,
                                     perf_mode=DR)
                    nc.tensor.matmul(rzp[:, j, :], whh_sb[:, :, j, :],
                                     h8[:, :, cs], start=False, stop=True,
                                     perf_mode=DR)
                for m in range(2):
                    nc.tensor.matmul(php[:, m, :], cb8_sb[:, :, m, :],
                                     oh_t[:, :, cs], start=True, stop=False,
                                     perf_mode=DR)
                    nc.tensor.matmul(php[:, m, :], whh_sb[:, :, 4 + m, :],
                                     h8[:, :, cs], start=False, stop=True,
                                     perf_mode=DR)

                if debug and t == 0:
                    prst = const.tile([128, 4, CH], f32, tag=f"tappr{c}",
                                      name=f"tappr{c}")
                    nc.vector.tensor_scalar(out=prst[:], in0=rzp[:], scalar1=1.0,
                                            scalar2=None, op0=Alu.mult)
                    nc.sync.dma_start(out=dbg_pr[:, :, cs], in_=prst[:])
                    phst = const.tile([128, 2, CH], f32, tag=f"tapph{c}",
                                      name=f"tapph{c}")
                    nc.vector.tensor_scalar(out=phst[:], in0=php[:], scalar1=1.0,
                                            scalar2=None, op0=Alu.mult)
                    nc.sync.dma_start(out=dbg_ph[:, :, cs], in_=phst[:])

                # gates: sigmoid over merged r/z psum (descale folded in)
                nc.scalar.activation(out=rzbf[:, :, cs], in_=rzp[:],
                                     func=Act.Sigmoid, scale=1.0 / S)
                # npre = phn * r  (b_hh_n already inside phn, S-scaled)
                nc.vector.tensor_tensor(
                    out=npre[:, :, cs], in0=php[:], in1=rzbf[:, 0:2, cs],
                    op=Alu.mult,
                )
                # t2 = npre + gin   (both S-scaled)
                nc.vector.tensor_add(t2[:, :, cs], npre[:, :, cs], gin[:, :, cs])

            for pp in range(2):
                ps_ = slice(pp * 2 * CH, (pp + 1) * 2 * CH)
                nc.scalar.activation(out=nbf[:, :, ps_], in_=t2[:, :, ps_],
                                     func=Act.Tanh, scale=1.0 / S)
                # h' = n + z*(h - n)
                nc.vector.tensor_sub(dbf[:, :, ps_], hbf[:, :, ps_], nbf[:, :, ps_])
                nc.vector.tensor_mul(ebf[:, :, ps_], rzbf[:, 2:4, ps_], dbf[:, :, ps_])
                nc.vector.tensor_add(hbf_new[:, :, ps_], nbf[:, :, ps_], ebf[:, :, ps_])
                nc.vector.tensor_scalar(out=h8_new[:, :, ps_], in0=hbf_new[:, :, ps_],
                                        scalar1=1.0, scalar2=None, op0=Alu.mult)

            if debug and t == 0:
                def tap(dst, src, nm):
                    st = const.tile(list(src.shape), f32, tag=f"tap{nm}", name=f"tap{nm}")
                    nc.vector.tensor_scalar(out=st[:], in0=src[:], scalar1=1.0,
                                            scalar2=None, op0=Alu.mult)
                    nc.sync.dma_start(out=dst[:], in_=st[:])
                tap(dbg_rz, rzbf, "rz")
                tap(dbg_npre, npre, "np")
                tap(dbg_t2, t2, "t2")
                tap(dbg_h1, hbf_new, "h1")
                tap(dbg_h0, hbf, "h0")
                tap(dbg_gin, gin, "gi")

            hbf = hbf_new
            h8 = h8_new
            emit_proj(h8, t)

    nc.finalize()
    return nc


def _quant_fp8(x, scale=1.0):
    return (np.asarray(x, dtype=np.float32) * scale).astype(FP8)


def _prep_inputs(latent, target, embed, W_ih, b_ih, W_hh, b_hh,
                 Wd0, bd0, Wd1, bd1, Wd2, bd2, Wp1, bp1, Wp2, bp2):
    f32 = np.float32
    latent = np.asarray(latent, dtype=f32)
    embed = np.asarray(embed, dtype=f32)
    W_ih = np.asarray(W_ih, dtype=f32)
    b_ih = np.asarray(b_ih, dtype=f32)
    W_hh = np.asarray(W_hh, dtype=f32)
    b_hh = np.asarray(b_hh, dtype=f32)

    # teacher-forcing shifted tokens, time-major
    tokens = np.concatenate(
        [np.zeros((B, 1), dtype=np.int64), np.asarray(target[:, :-1], dtype=np.int64)],
        axis=1,
    )  # [B, T]
    tok_tm = tokens.T  # [T, B]

    # one-hot, vocab split into 2 k-tiles of KT (A=32 + const-1 + pad = 34)
    ohf = np.zeros((T, KT, 2, B), dtype=FP8)
    for a in range(A):
        i, k = divmod(a, KT)
        ohf[:, k, i, :][tok_tm == a] = 1.0
    ohf[:, 16, 1, :] = 1.0  # const row (A=32 -> i=1,k=15 used; 16 is const)

    # NOTE: vocab rows: a in [0,16] -> tile0 row a; a in [17,31] -> tile1 row a-17
    # const row: tile1 row 16 (since 32 vocab + const = 33 <= 34)

    giv = embed @ W_ih.T  # [A, 3H]
    brow_rz = (b_ih + b_hh)[: 2 * H]  # r/z biases folded into const row

    # giv8: [KT, 2, 4, 128] for r/z feature blocks
    giv8 = np.zeros((KT, 2, 4, 128), dtype=FP8)
    for a in range(A):
        i, k = divmod(a, KT)
        giv8[k, i] = _quant_fp8(giv[a, : 2 * H].reshape(4, 128), S)
    giv8[16, 1] = _quant_fp8(brow_rz.reshape(4, 128), S)

    # cb8: const-row-only table injecting S*b_hh_n into phn
    cb8 = np.zeros((KT, 2, 2, 128), dtype=FP8)
    cb8[16, 1] = _quant_fp8(b_hh[2 * H:].reshape(2, 128), S)

    # whh8: [128, 2, 6, 128] = S*W_hh.T split k-tiles x feature blocks
    whhT = np.ascontiguousarray(W_hh.T)  # [H, 3H]
    whh8 = np.ascontiguousarray(
        whhT.reshape(2, 128, 6, 128).transpose(1, 0, 2, 3))
    whh8 = _quant_fp8(whh8, S)

    # MLP weights bf16
    wd0_l = np.ascontiguousarray(
        np.asarray(Wd0, dtype=f32).reshape(128, 2, 128)).astype(BF16)
    wd1_l = np.ascontiguousarray(
        np.asarray(Wd1, dtype=f32).reshape(2, 128, 2, 128).transpose(1, 0, 2, 3)).astype(BF16)
    wd2_l = np.ascontiguousarray(
        np.asarray(Wd2, dtype=f32).reshape(2, 128, 2, 128).transpose(1, 0, 2, 3)).astype(BF16)

    # projections
    Sp = 16.0
    wp18 = np.ascontiguousarray(
        np.asarray(Wp1, dtype=f32).reshape(2, 128, A).transpose(1, 0, 2))
    wp18 = _quant_fp8(wp18, Sp)
    wp2_l = np.ascontiguousarray(
        np.tile(np.asarray(Wp2, dtype=f32) / Sp, (4, 1))).astype(BF16)  # [128, 32]
    bp2_l = np.asarray(bp2, dtype=f32)[None, :].astype(BF16)  # [1, 32]

    bias_pack = np.zeros((128, 8), dtype=f32)
    bias_pack[:, 0] = np.asarray(bd0, dtype=f32)[:128]
    bias_pack[:, 1] = np.asarray(bd0, dtype=f32)[128:]
    bias_pack[:, 2] = np.asarray(bd1, dtype=f32)[:128]
    bias_pack[:, 3] = np.asarray(bd1, dtype=f32)[128:]
    bias_pack[:, 4] = np.asarray(bd2, dtype=f32)[:128]
    bias_pack[:, 5] = np.asarray(bd2, dtype=f32)[128:]
    bias_pack[:, 6] = np.tile(np.asarray(bp1, dtype=f32) * Sp, 4)

    # n-gate gather table: S*(giv_n.T + b_ih_n)
    givT_n = giv.T[2 * H:] + b_ih[2 * H:, None]  # [256, 32]
    gtab = np.ascontiguousarray(
        (givT_n * S).reshape(2, 128, A).transpose(1, 0, 2)).astype(BF16)

    latT = np.ascontiguousarray(latent.T).astype(BF16)  # [128, B]

    shared = dict(whh=whh8, giv=giv8, cb8=cb8, wd0=wd0_l, wd1=wd1_l,
                  wd2=wd2_l, wp18=wp18, wp2=wp2_l, bp2r=bp2_l,
                  bias=bias_pack, gtab=gtab)
    in_maps = []
    for c in range(NCORES):
        bs = slice(c * BC, (c + 1) * BC)
        m = dict(shared)
        m["lat"] = np.ascontiguousarray(latT[:, bs])
        m["oh"] = np.ascontiguousarray(ohf[:, :, :, bs])
        tok_c = tokens[bs]                               # [1024, T]
        w = tok_c.reshape(64, 16, T).transpose(1, 2, 0)  # [16, T, 64]
        m["tokw"] = np.ascontiguousarray(
            np.tile(w, (8, 1, 1))).astype(np.uint16)     # [128, T, 64]
        in_maps.append(m)
    return in_maps


def kernel(**inputs):
    from concourse.bass_utils import run_bass_kernel_spmd

    if "nc" not in _CACHE:
        _CACHE["nc"] = _build()
    nc = _CACHE["nc"]

    in_maps = _prep_inputs(**inputs)
    res = run_bass_kernel_spmd(nc, in_maps, core_ids=list(range(NCORES)))
    outs = [r["out"] for r in res.results]
    return np.concatenate(outs, axis=0).astype(np.float32)


# revision 33
# speedup vs baseline: 1.2721x; 1.2721x over previous
"""GRU decoder (nn_Decoder) Trainium2 Bass kernel — fp8 DoubleRow edition.

Strategy: pure data parallelism — batch B=8192 over 8 cores (1024 rows each),
weights replicated. Features on partitions, batch on the free axis.

Per core, per GRU step (batch chunks of N=256):
  - All gate matmuls are fp8e4 DoubleRow (0.5 cy/row): recurrent W_hh.T
    (scaled by S, quantized to fp8) against h8 (fp8 copy of h), plus one-hot
    token matmuls for the r/z input gates (table pre-scaled by S, biases
    folded into a const-1 row). The n-gate input row is gathered on GPSIMD
    from a [128,2,A] table (pre-scaled by S, b_ih_n folded); b_hh_n enters
    the phn PSUM via a const-row matmul so npre is a plain tensor_tensor.
  - Act: sigmoid over the merged [128,4,256] r/z PSUM (scale=1/S), tanh over
    the SBUF t2 tensor (scale=1/S), one table, no reloads.
  - GPSIMD: npre = phn*r (stt, PSUM in), token gathers.
  - DVE: t2 = npre + gin, combine h' = n + z*(h-n), h8 convert, p1 relu.
  - Projections p1/p2 fp8/bf16 with batch-packed PSUM [128, 512]; logits
    (+bp2 via a const-row matmul) DMA'd straight from PSUM to DRAM.
"""

import numpy as np
import ml_dtypes

B, L, H, A, T, E = 8192, 128, 256, 32, 65, 8
NCORES = 8
BC = B // NCORES          # 1024 batch rows per core
NCH = 4                   # batch chunks per step
CH = BC // NCH            # 256
G3 = 3 * H                # 768
S = 16.0                  # fp8 table scale
KT = 17                   # one-hot k-tile height (2*17 = 34 = A + const + pad)

BF16 = ml_dtypes.bfloat16
FP8 = ml_dtypes.float8_e4m3fn

_CACHE = {}


def _build(trace=False, debug=False):
    import concourse.bass as bass
    import concourse.bacc as bacc
    import concourse.tile as tile
    from concourse import mybir
    from contextlib import ExitStack

    f32 = mybir.dt.float32
    bf16 = mybir.dt.bfloat16
    fp8 = mybir.dt.float8e4
    u16 = mybir.dt.uint16
    Alu = mybir.AluOpType
    Act = mybir.ActivationFunctionType
    DR = mybir.MatmulPerfMode.DoubleRow

    nc = bacc.Bacc("TRN2", target_bir_lowering=False, debug=False)

    lat = nc.dram_tensor("lat", [128, BC], bf16, kind="ExternalInput")
    oh = nc.dram_tensor("oh", [T, KT, 2, BC], fp8, kind="ExternalInput")
    whh = nc.dram_tensor("whh", [128, 2, 6, 128], fp8, kind="ExternalInput")
    giv = nc.dram_tensor("giv", [KT, 2, 4, 128], fp8, kind="ExternalInput")
    cb8 = nc.dram_tensor("cb8", [KT, 2, 2, 128], fp8, kind="ExternalInput")
    wd0 = nc.dram_tensor("wd0", [128, 2, 128], bf16, kind="ExternalInput")
    wd1 = nc.dram_tensor("wd1", [128, 2, 2, 128], bf16, kind="ExternalInput")
    wd2 = nc.dram_tensor("wd2", [128, 2, 2, 128], bf16, kind="ExternalInput")
    wp18 = nc.dram_tensor("wp18", [128, 2, A], fp8, kind="ExternalInput")
    wp2 = nc.dram_tensor("wp2", [128, A], bf16, kind="ExternalInput")
    bp2r = nc.dram_tensor("bp2r", [1, A], bf16, kind="ExternalInput")
    bias = nc.dram_tensor("bias", [128, 8], f32, kind="ExternalInput")
    tokw = nc.dram_tensor("tokw", [128, T, 64], u16, kind="ExternalInput")
    gtab = nc.dram_tensor("gtab", [128, 2, A], bf16, kind="ExternalInput")
    out = nc.dram_tensor("out", [BC, T, A], f32, kind="ExternalOutput")
    if debug:
        dbg_rz = nc.dram_tensor("dbg_rz", [128, 4, BC], f32, kind="ExternalOutput")
        dbg_npre = nc.dram_tensor("dbg_npre", [128, 2, BC], f32, kind="ExternalOutput")
        dbg_t2 = nc.dram_tensor("dbg_t2", [128, 2, BC], f32, kind="ExternalOutput")
        dbg_h1 = nc.dram_tensor("dbg_h1", [128, 2, BC], f32, kind="ExternalOutput")
        dbg_h0 = nc.dram_tensor("dbg_h0", [128, 2, BC], f32, kind="ExternalOutput")
        dbg_gin = nc.dram_tensor("dbg_gin", [128, 2, BC], f32, kind="ExternalOutput")
        dbg_p1 = nc.dram_tensor("dbg_p1", [A, BC], f32, kind="ExternalOutput")
        dbg_pr = nc.dram_tensor("dbg_pr", [128, 4, BC], f32, kind="ExternalOutput")
        dbg_ph = nc.dram_tensor("dbg_ph", [128, 2, BC], f32, kind="ExternalOutput")

    outv = out.rearrange("(j p) t a -> p j t a", j=8, p=128)

    with ExitStack() as ctx:
        tc = ctx.enter_context(tile.TileContext(nc))
        const = ctx.enter_context(tc.tile_pool(name="const", bufs=1))
        hp = ctx.enter_context(tc.tile_pool(name="hp", bufs=3))
        work = ctx.enter_context(tc.tile_pool(name="work", bufs=3))
        ohp = ctx.enter_context(tc.tile_pool(name="ohp", bufs=3))
        psum = ctx.enter_context(tc.tile_pool(name="psum", bufs=1, space="PSUM"))

        # ---- constants ----
        lat_sb = const.tile([128, BC], bf16, tag="lat")
        nc.sync.dma_start(out=lat_sb[:], in_=lat[:])
        whh_sb = const.tile([128, 2, 6, 128], fp8, tag="whh")
        nc.sync.dma_start(out=whh_sb[:], in_=whh[:])
        giv_sb = const.tile([KT, 2, 4, 128], fp8, tag="giv")
        nc.sync.dma_start(out=giv_sb[:], in_=giv[:])
        cb8_sb = const.tile([KT, 2, 2, 128], fp8, tag="cb8")
        nc.sync.dma_start(out=cb8_sb[:], in_=cb8[:])
        wd0_sb = const.tile([128, 2, 128], bf16, tag="wd0")
        nc.sync.dma_start(out=wd0_sb[:], in_=wd0[:])
        wd1_sb = const.tile([128, 2, 2, 128], bf16, tag="wd1")
        nc.sync.dma_start(out=wd1_sb[:], in_=wd1[:])
        wd2_sb = const.tile([128, 2, 2, 128], bf16, tag="wd2")
        nc.sync.dma_start(out=wd2_sb[:], in_=wd2[:])
        wp1_sb = const.tile([128, 2, A], fp8, tag="wp18")
        nc.sync.dma_start(out=wp1_sb[:], in_=wp18[:])
        wp2_sb = const.tile([128, A], bf16, tag="wp2")
        nc.sync.dma_start(out=wp2_sb[:], in_=wp2[:])
        bp2_sb = const.tile([1, A], bf16, tag="bp2r")
        nc.sync.dma_start(out=bp2_sb[:], in_=bp2r[:])
        one_sb = const.tile([1, 128], bf16, tag="one")
        nc.vector.memset(one_sb[:], 1.0)
        bias_sb = const.tile([128, 8], f32, tag="bias")
        nc.sync.dma_start(out=bias_sb[:], in_=bias[:])
        tokw_sb = const.tile([128, T, 64], u16, tag="tokw")
        nc.sync.dma_start(out=tokw_sb[:], in_=tokw[:])
        gtab_sb = const.tile([128, 2, A], bf16, tag="gtab")
        nc.sync.dma_start(out=gtab_sb[:], in_=gtab[:])

        # ---- MLP prologue: h0 = relu(relu(lat@Wd0+b)@Wd1+b)@Wd2+b ----
        # bf16 matmuls, feature-on-partition layout, chunked by CH columns.
        h1 = work.tile([128, 2, BC], bf16, tag="mlp1", name="mlp1")
        for c in range(NCH):
            cs = slice(c * CH, (c + 1) * CH)
            ps = psum.tile([128, 2, CH], f32, tag="ph", bufs=1, name=f"mlp1p_{c}")
            for m in range(2):
                nc.tensor.matmul(ps[:, m, :], wd0_sb[:, m, :], lat_sb[:, cs],
                                 start=True, stop=True)
            for m in range(2):
                nc.scalar.activation(out=h1[:, m, cs], in_=ps[:, m, :],
                                     func=Act.Relu, bias=bias_sb[:, m:m + 1])
        h2 = work.tile([128, 2, BC], bf16, tag="mlp2", name="mlp2")
        for c in range(NCH):
            cs = slice(c * CH, (c + 1) * CH)
            ps = psum.tile([128, 2, CH], f32, tag="ph", bufs=1, name=f"mlp2p_{c}")
            for m in range(2):
                for kc in range(2):
                    nc.tensor.matmul(ps[:, m, :], wd1_sb[:, kc, m, :],
                                     h1[:, kc, cs], start=(kc == 0), stop=(kc == 1))
            for m in range(2):
                nc.scalar.activation(out=h2[:, m, cs], in_=ps[:, m, :],
                                     func=Act.Relu, bias=bias_sb[:, 2 + m:3 + m])
        hbf = hp.tile([128, 2, BC], bf16, tag="hbf", name="h0bf")
        h8 = hp.tile([128, 2, BC], fp8, tag="h8", name="h0f8")
        for c in range(NCH):
            cs = slice(c * CH, (c + 1) * CH)
            ps = psum.tile([128, 2, CH], f32, tag="ph", bufs=1, name=f"mlp3p_{c}")
            for m in range(2):
                for kc in range(2):
                    nc.tensor.matmul(ps[:, m, :], wd2_sb[:, kc, m, :],
                                     h2[:, kc, cs], start=(kc == 0), stop=(kc == 1))
            for m in range(2):
                nc.scalar.activation(out=hbf[:, m, cs], in_=ps[:, m, :],
                                     func=Act.Identity, bias=bias_sb[:, 4 + m:5 + m])
        nc.vector.tensor_scalar(out=h8[:], in0=hbf[:], scalar1=1.0,
                                scalar2=None, op0=Alu.mult)

        # ---- GRU steps ----
        def emit_p1(h8_t, tp):
            """p1 matmuls + relu for step tp (emitted late in step tp+1)."""
            p1ps = psum.tile([A, BC], f32, tag="p1", bufs=1, name=f"p1ps_{tp}")
            for cc in range(NCH):
                nc.tensor.matmul(
                    p1ps[:, cc * CH:(cc + 1) * CH], wp1_sb[:],
                    h8_t[:, :, cc * CH:(cc + 1) * CH],
                    start=True, stop=True, perf_mode=DR,
                )
            p1bf = work.tile([A, BC], bf16, tag="p1bf", name=f"p1bf_{tp}")
            nc.scalar.activation(out=p1bf[:], in_=p1ps[:], func=Act.Relu,
                                 bias=bias_sb[0:A, 6:7])
            return p1bf

        def emit_p2(p1bf, tp):
            """p2 matmuls + staging + output DMA for step tp."""
            p2v = psum.tile([128, 8, A], f32, tag="p2", bufs=1, name=f"p2ps_{tp}")
            for j in range(8):
                nc.tensor.matmul(
                    p2v[:, j, :], one_sb[:], bp2_sb[:],
                    start=True, stop=False, tile_position=(0, 0),
                )
                nc.tensor.matmul(
                    p2v[:, j, :], p1bf[:, 128 * j:128 * (j + 1)], wp2_sb[0:A, :],
                    start=False, stop=True,
                )
            outsb = work.tile([128, 8, A], f32, tag="outsb", name=f"outsb_{tp}")
            nc.scalar.activation(out=outsb[:], in_=p2v[:], func=Act.Copy)
            nc.sync.dma_start(out=outv[:, :, tp, :], in_=outsb[:])

        prev_h8 = None
        prev2_p1bf = None
        for t in range(T):
            oh_t = ohp.tile([KT, 2, BC], fp8, tag="oh", name=f"oh_{t}")
            nc.sync.dma_start(out=oh_t[:], in_=oh[t])
            # n-gate input rows gathered by token on GPSIMD (S-scaled table)
            gin = work.tile([128, 2, BC], bf16, tag="gin", name=f"gin_{t}")
            for m in range(2):
                nc.gpsimd.indirect_copy(
                    out=gin[:, m, :], data=gtab_sb[:, m, :],
                    idxs=tokw_sb[:, t, :],
                    i_know_ap_gather_is_preferred=True,
                )

            rzbf = work.tile([128, 4, BC], bf16, tag="rzbf", name=f"rzbf_{t}")
            npre = work.tile([128, 2, BC], bf16, tag="npre", name=f"npre_{t}")
            t2 = work.tile([128, 2, BC], bf16, tag="t2", name=f"t2_{t}")
            nbf = work.tile([128, 2, BC], bf16, tag="nbf", name=f"nbf_{t}")
            dbf = work.tile([128, 2, BC], bf16, tag="dbf", name=f"dbf_{t}")
            ebf = work.tile([128, 2, BC], bf16, tag="ebf", name=f"ebf_{t}")
            hbf_new = hp.tile([128, 2, BC], bf16, tag="hbf", name=f"hbf_{t}")
            h8_new = hp.tile([128, 2, BC], fp8, tag="h8", name=f"h8_{t}")

            def onehots(c):
                """One-hot/const matmuls for chunk c — no h8 dependency, can
                run far ahead. Bankfirst starts: one start=True per 2KB bank."""
                cs = slice(c * CH, (c + 1) * CH)
                rzp = psum.tile([128, 4, CH], f32, tag="rz", bufs=2,
                                name=f"rzp_{t}_{c}")
                php = psum.tile([128, 2, CH], f32, tag="ph", bufs=1,
                                name=f"php_{t}_{c}")
                for j in range(4):
                    nc.tensor.matmul(rzp[:, j, :], giv_sb[:, :, j, :],
                                     oh_t[:, :, cs], start=(j % 2 == 0),
                                     stop=False, perf_mode=DR,
                                     skip_group_check=True)
                nc.tensor.matmul(php[:, 0, :], cb8_sb[:, :, 0, :],
                                 oh_t[:, :, cs], start=True, stop=False,
                                 perf_mode=DR, skip_group_check=True)
                nc.tensor.matmul(php[:, 1, :], cb8_sb[:, :, 1, :],
                                 oh_t[:, :, cs], start=False, stop=False,
                                 perf_mode=DR, skip_group_check=True)
                return rzp, php

            def gates(c, rzp, php):
                """Recurrent matmuls + sigmoid + n-path front for chunk c."""
                cs = slice(c * CH, (c + 1) * CH)
                for j in range(4):
                    nc.tensor.matmul(rzp[:, j, :], whh_sb[:, :, j, :],
                                     h8[:, :, cs], start=False, stop=True,
                                     perf_mode=DR, skip_group_check=True)
                for m in range(2):
                    nc.tensor.matmul(php[:, m, :], whh_sb[:, :, 4 + m, :],
                                     h8[:, :, cs], start=False, stop=True,
                                     perf_mode=DR, skip_group_check=True)
                # gates: sigmoid over merged r/z psum (descale folded in)
                nc.scalar.activation(out=rzbf[:, :, cs], in_=rzp[:],
                                     func=Act.Sigmoid, scale=1.0 / S)
                # npre = phn * r  (b_hh_n already inside phn, S-scaled)
                nc.vector.tensor_tensor(
                    out=npre[:, :, cs], in0=php[:], in1=rzbf[:, 0:2, cs],
                    op=Alu.mult,
                )
                # t2 = npre + gin   (both S-scaled)
                nc.vector.tensor_add(t2[:, :, cs], npre[:, :, cs], gin[:, :, cs])

            def tail(ps_):
                """tanh + combine + h8 for a batch slice."""
                nc.scalar.activation(out=nbf[:, :, ps_], in_=t2[:, :, ps_],
                                     func=Act.Tanh, scale=1.0 / S)
                # h' = n + z*(h - n)
                nc.vector.tensor_sub(dbf[:, :, ps_], hbf[:, :, ps_], nbf[:, :, ps_])
                nc.vector.tensor_mul(ebf[:, :, ps_], rzbf[:, 2:4, ps_], dbf[:, :, ps_])
                nc.vector.tensor_add(hbf_new[:, :, ps_], nbf[:, :, ps_], ebf[:, :, ps_])
                nc.vector.tensor_scalar(out=h8_new[:, :, ps_], in0=hbf_new[:, :, ps_],
                                        scalar1=1.0, scalar2=None, op0=Alu.mult)

            # A: drain step t-2's projections (everything ready)
            if prev2_p1bf is not None:
                emit_p2(prev2_p1bf, t - 2)
            # B: all one-hot matmuls up front (PE never blocks on them)
            ps_tiles = [onehots(c) for c in range(NCH)]
            # C: per-chunk recurrent work with interleaved tails
            gates(0, *ps_tiles[0])
            gates(1, *ps_tiles[1])
            tail(slice(0, CH))
            gates(2, *ps_tiles[2])
            tail(slice(CH, 2 * CH))
            gates(3, *ps_tiles[3])
            tail(slice(2 * CH, BC))
            # D: step t-1's p1 + relu (h8(t-1) long ready; runs when PE drains)
            if prev_h8 is not None:
                prev2_p1bf = emit_p1(prev_h8, t - 1)
            prev_h8 = h8_new

            hbf = hbf_new
            h8 = h8_new
        # epilogue: flush the two lagged projection stages
        emit_p2(prev2_p1bf, T - 2)
        last_p1bf = emit_p1(prev_h8, T - 1)
        emit_p2(last_p1bf, T - 1)

    nc.finalize()
    return nc


def _quant_fp8(x, scale=1.0):
    return (np.asarray(x, dtype=np.float32) * scale).astype(FP8)


def _prep_inputs(latent, target, embed, W_ih, b_ih, W_hh, b_hh,
                 Wd0, bd0, Wd1, bd1, Wd2, bd2, Wp1, bp1, Wp2, bp2):
    f32 = np.float32
    latent = np.asarray(latent, dtype=f32)
    embed = np.asarray(embed, dtype=f32)
    W_ih = np.asarray(W_ih, dtype=f32)
    b_ih = np.asarray(b_ih, dtype=f32)
    W_hh = np.asarray(W_hh, dtype=f32)
    b_hh = np.asarray(b_hh, dtype=f32)

    # teacher-forcing shifted tokens, time-major
    tokens = np.concatenate(
        [np.zeros((B, 1), dtype=np.int64), np.asarray(target[:, :-1], dtype=np.int64)],
        axis=1,
    )  # [B, T]
    tok_tm = tokens.T  # [T, B]

    # one-hot, vocab split into 2 k-tiles of KT (A=32 + const-1 + pad = 34)
    ohf = np.zeros((T, KT, 2, B), dtype=FP8)
    for a in range(A):
        i, k = divmod(a, KT)
        ohf[:, k, i, :][tok_tm == a] = 1.0
    ohf[:, 16, 1, :] = 1.0  # const row (A=32 -> i=1,k=15 used; 16 is const)

    # NOTE: vocab rows: a in [0,16] -> tile0 row a; a in [17,31] -> tile1 row a-17
    # const row: tile1 row 16 (since 32 vocab + const = 33 <= 34)

    giv = embed @ W_ih.T  # [A, 3H]
    brow_rz = (b_ih + b_hh)[: 2 * H]  # r/z biases folded into const row

    # giv8: [KT, 2, 4, 128] for r/z feature blocks
    giv8 = np.zeros((KT, 2, 4, 128), dtype=FP8)
    for a in range(A):
        i, k = divmod(a, KT)
        giv8[k, i] = _quant_fp8(giv[a, : 2 * H].reshape(4, 128), S)
    giv8[16, 1] = _quant_fp8(brow_rz.reshape(4, 128), S)

    # cb8: const-row-only table injecting S*b_hh_n into phn
    cb8 = np.zeros((KT, 2, 2, 128), dtype=FP8)
    cb8[16, 1] = _quant_fp8(b_hh[2 * H:].reshape(2, 128), S)

    # whh8: [128, 2, 6, 128] = S*W_hh.T split k-tiles x feature blocks
    whhT = np.ascontiguousarray(W_hh.T)  # [H, 3H]
    whh8 = np.ascontiguousarray(
        whhT.reshape(2, 128, 6, 128).transpose(1, 0, 2, 3))
    whh8 = _quant_fp8(whh8, S)

    # MLP weights bf16
    wd0_l = np.ascontiguousarray(
        np.asarray(Wd0, dtype=f32).reshape(128, 2, 128)).astype(BF16)
    wd1_l = np.ascontiguousarray(
        np.asarray(Wd1, dtype=f32).reshape(2, 128, 2, 128).transpose(1, 0, 2, 3)).astype(BF16)
    wd2_l = np.ascontiguousarray(
        np.asarray(Wd2, dtype=f32).reshape(2, 128, 2, 128).transpose(1, 0, 2, 3)).astype(BF16)

    # projections
    Sp = 16.0
    wp18 = np.ascontiguousarray(
        np.asarray(Wp1, dtype=f32).reshape(2, 128, A).transpose(1, 0, 2))
    wp18 = _quant_fp8(wp18, Sp)
    wp2_l = np.ascontiguousarray(
        np.tile(np.asarray(Wp2, dtype=f32) / Sp, (4, 1))).astype(BF16)  # [128, 32]
    bp2_l = np.asarray(bp2, dtype=f32)[None, :].astype(BF16)  # [1, 32]

    bias_pack = np.zeros((128, 8), dtype=f32)
    bias_pack[:, 0] = np.asarray(bd0, dtype=f32)[:128]
    bias_pack[:, 1] = np.asarray(bd0, dtype=f32)[128:]
    bias_pack[:, 2] = np.asarray(bd1, dtype=f32)[:128]
    bias_pack[:, 3] = np.asarray(bd1, dtype=f32)[128:]
    bias_pack[:, 4] = np.asarray(bd2, dtype=f32)[:128]
    bias_pack[:, 5] = np.asarray(bd2, dtype=f32)[128:]
    bias_pack[:, 6] = np.tile(np.asarray(bp1, dtype=f32) * Sp, 4)

    # n-gate gather table: S*(giv_n.T + b_ih_n)
    givT_n = giv.T[2 * H:] + b_ih[2 * H:, None]  # [256, 32]
    gtab = np.ascontiguousarray(
        (givT_n * S).reshape(2, 128, A).transpose(1, 0, 2)).astype(BF16)

    latT = np.ascontiguousarray(latent.T).astype(BF16)  # [128, B]

    shared = dict(whh=whh8, giv=giv8, cb8=cb8, wd0=wd0_l, wd1=wd1_l,
                  wd2=wd2_l, wp18=wp18, wp2=wp2_l, bp2r=bp2_l,
                  bias=bias_pack, gtab=gtab)
    in_maps = []
    for c in range(NCORES):
        bs = slice(c * BC, (c + 1) * BC)
        m = dict(shared)
        m["lat"] = np.ascontiguousarray(latT[:, bs])
        m["oh"] = np.ascontiguousarray(ohf[:, :, :, bs])
        tok_c = tokens[bs]                               # [1024, T]
        w = tok_c.reshape(64, 16, T).transpose(1, 2, 0)  # [16, T, 64]
        m["tokw"] = np.ascontiguousarray(
            np.tile(w, (8, 1, 1))).astype(np.uint16)     # [128, T, 64]
        in_maps.append(m)
    return in_maps


def kernel(**inputs):
    from concourse.bass_utils import run_bass_kernel_spmd

    if "nc" not in _CACHE:
        _CACHE["nc"] = _build()
    nc = _CACHE["nc"]

    in_maps = _prep_inputs(**inputs)
    res = run_bass_kernel_spmd(nc, in_maps, core_ids=list(range(NCORES)))
    outs = [r["out"] for r in res.results]
    return np.concatenate(outs, axis=0).astype(np.float32)
